# revision 10
# baseline (speedup 1.0000x reference)
"""Trainium2 Bass kernel for the attention module (b=4, c=256, l=2048, h=8, d=64).

Sharding: 8 cores = 4 batches x 2 query-halves. Each core receives its batch's
x with columns permuted so its own query half comes first; it computes k/v for
all 2048 key positions and the attention output for its 1024 queries, then the
output projection + final rms-norm for its slice. Output slices are disjoint,
so no collectives are needed and the host just reassembles.

Device algorithm per core (all fp32, matmuls in fp32r):
  ss_j   = sum_c x[c,j]^2                      (PE ones-reduce)
  s_j    = 1/sqrt(ss_j/256 + eps)              (ACT sqrt + DVE reciprocal)
  xn     = x * bcast(s)                        (PE ones-broadcast + DVE)
  q,k    = (wqkvT-slices).T @ xn               (g1, 1/sqrt(d) folded on host)
  vT     = xn_slice.T @ wqkvT[:,2H:3H]         (v transposed, + ones column)
  per head:  simT = k_h.T @ q_h  (j on partitions, i free)
             p = exp(simT)                     (no max-subtract: logits bounded)
             [y_h; den] += [v_h | 1].T @ p     (denominator fused as column 65)
  y_h   /= bcast(den_h)                        (colsel matmul bcast + DVE)
  out    = woutT.T @ y + b_out
  result = out * bcast(1/sqrt(ss2/256+eps)) * g2
"""
import os
import sys

import numpy as np

if "/opt/trn_rl_repo" not in sys.path:
    sys.path.insert(0, "/opt/trn_rl_repo")

import concourse.bass as bass  # noqa: E402
import concourse.tile as tile  # noqa: E402
from concourse import bacc, mybir  # noqa: E402
from concourse.bass_utils import run_bass_kernel_spmd  # noqa: E402

F32 = mybir.dt.float32
F32R = mybir.dt.float32r
AF = mybir.ActivationFunctionType

B, C, L = 4, 256, 2048
H, D = 8, 64
HID = H * D
LQ = L // 2      # queries per core
NCT = C // 128   # 2 c-tiles
NJ = L // 128    # 16 j-tiles
EPS_B = 1e-26    # bias inside sqrt(ss/256 + eps): clamps like max(n,1e-12)


def _r(ap):
    return ap.bitcast(F32R)


def _body(tc, x, wq, wo, bo, g2, out):
    nc = tc.nc
    from contextlib import ExitStack
    with ExitStack() as ctx:
        ctx.enter_context(nc.allow_low_precision(
            reason="fp32r matmul operands are rounded by design"))
        const = ctx.enter_context(tc.tile_pool(name="const", bufs=1))
        big = ctx.enter_context(tc.tile_pool(name="big", bufs=4))
        xnp = ctx.enter_context(tc.tile_pool(name="xn", bufs=2))
        qp = ctx.enter_context(tc.tile_pool(name="q", bufs=4))
        vtp = ctx.enter_context(tc.tile_pool(name="vt", bufs=16))
        ptp = ctx.enter_context(tc.tile_pool(name="pt", bufs=3))
        yp = ctx.enter_context(tc.tile_pool(name="y", bufs=4))
        invp = ctx.enter_context(tc.tile_pool(name="inv", bufs=1))
        widep = ctx.enter_context(tc.tile_pool(name="wide", bufs=2))
        outp = ctx.enter_context(tc.tile_pool(name="outp", bufs=2))
        finp = ctx.enter_context(tc.tile_pool(name="fin", bufs=2))

        # ---------------- constants & weights ----------------
        # memset cannot write float32r; stage in f32 and copy (dtype-convert).
        stage_col = const.tile([128, 8], F32, tag="stage_col")
        nc.vector.memset(stage_col[:], 1.0)
        stage_row = const.tile([1, 128], F32, tag="stage_row")
        nc.vector.memset(stage_row[:], 1.0)
        ones_col = const.tile([128, 1], F32R, tag="ones_col")
        nc.vector.tensor_copy(ones_col[:], stage_col[:, 0:1])
        ones_row = const.tile([1, 128], F32R, tag="ones_row")
        nc.vector.tensor_copy(ones_row[:], stage_row[:])
        eps_t = const.tile([1, 1], F32, tag="eps")
        nc.vector.memset(eps_t[:], EPS_B)
        stage_sel = []
        for i in range(2):
            t = const.tile([1, 128], F32, tag=f"stage_sel{i}")
            nc.vector.memset(t[:, 0:64], 1.0 if i == 0 else 0.0)
            nc.vector.memset(t[:, 64:128], 0.0 if i == 0 else 1.0)
            stage_sel.append(t)
        colsel = []
        for i in range(2):
            t = const.tile([1, 128], F32R, tag=f"colsel{i}")
            nc.vector.tensor_copy(t[:], stage_sel[i][:])
            colsel.append(t)

        wq_sb = []
        for ct in range(NCT):
            t = const.tile([128, 3 * HID], F32R, tag=f"wq{ct}")
            nc.sync.dma_start(t[:], wq[ct * 128:(ct + 1) * 128, :])
            wq_sb.append(t)
        wo_sb = []
        for kt in range(4):
            t = const.tile([128, C], F32R, tag=f"wo{kt}")
            nc.sync.dma_start(t[:], wo[kt * 128:(kt + 1) * 128, :])
            wo_sb.append(t)
        bo_sb, g2_sb = [], []
        for mt in range(2):
            tb = const.tile([128, 1], F32, tag=f"bo{mt}")
            nc.sync.dma_start(tb[:], bo[mt * 128:(mt + 1) * 128, :])
            bo_sb.append(tb)
            tg = const.tile([128, 1], F32, tag=f"g2{mt}")
            nc.sync.dma_start(tg[:], g2[mt * 128:(mt + 1) * 128, :])
            g2_sb.append(tg)

        x_sb = []
        for ct in range(NCT):
            t = big.tile([128, L], F32, tag="big")
            nc.sync.dma_start(t[:], x[ct * 128:(ct + 1) * 128, :])
            x_sb.append(t)

        # ---------------- input rms-norm ----------------
        with tc.tile_pool(name="ps_pre", bufs=1, space="PSUM") as ps_pre:
            ss = ps_pre.tile([1, L], F32, tag="ss")
            for n in range(L // 512):
                for ct in range(NCT):
                    sq = ptp.tile([128, 512], F32R, tag="pt")
                    xs = x_sb[ct][:, n * 512:(n + 1) * 512]
                    nc.vector.tensor_mul(sq[:], xs, xs)
                    nc.tensor.matmul(ss[0:1, n * 512:(n + 1) * 512],
                                     (ones_col[:]), (sq[:]),
                                     start=(ct == 0), stop=(ct == NCT - 1))
            stmp = widep.tile([1, L], F32, tag="wide")
            nc.scalar.activation(stmp[:], ss[0:1, :], AF.Sqrt,
                                 bias=eps_t[:], scale=1.0 / C)
            s1 = widep.tile([1, L], F32R, tag="wide")
            nc.vector.reciprocal(s1[:], stmp[:])
            bc_in = ps_pre.tile([128, L], F32, tag="bc_in")
            for n in range(L // 512):
                nc.tensor.matmul(bc_in[:, n * 512:(n + 1) * 512],
                                 (ones_row[:]), (s1[0:1, n * 512:(n + 1) * 512]),
                                 start=True, stop=True)
            xn_sb = []
            for ct in range(NCT):
                t = xnp.tile([128, L], F32R, tag="xn")
                nc.vector.tensor_mul(t[:], x_sb[ct][:], bc_in[:, :])
                xn_sb.append(t)

        # ---------------- qkv projections ----------------
        q_sb, k_sb, vt_sb = [], [], []
        with tc.tile_pool(name="ps_mm", bufs=3, space="PSUM") as ps_mm:
            for mt in range(4):       # q: rows mt*128, my queries only
                ps = ps_mm.tile([128, LQ], F32, tag="mm")
                for n in range(LQ // 512):
                    for ct in range(NCT):
                        nc.tensor.matmul(
                            ps[:, n * 512:(n + 1) * 512],
                            (wq_sb[ct][:, mt * 128:(mt + 1) * 128]),
                            (xn_sb[ct][:, n * 512:(n + 1) * 512]),
                            start=(ct == 0), stop=(ct == NCT - 1))
                t = qp.tile([128, LQ], F32R, tag="q")
                nc.vector.tensor_copy(t[:], ps[:, :])
                q_sb.append(t)
            for mt in range(4):       # k: rows 512+mt*128, all keys
                t = big.tile([128, L], F32R, tag="big")
                for half in range(2):
                    ps = ps_mm.tile([128, LQ], F32, tag="mm")
                    for n in range(2):
                        nl = half * 2 + n
                        for ct in range(NCT):
                            nc.tensor.matmul(
                                ps[:, n * 512:(n + 1) * 512],
                                (wq_sb[ct][:, HID + mt * 128:HID + (mt + 1) * 128]),
                                (xn_sb[ct][:, nl * 512:(nl + 1) * 512]),
                                start=(ct == 0), stop=(ct == NCT - 1))
                    nc.vector.tensor_copy(t[:, half * LQ:(half + 1) * LQ], ps[:, :])
                k_sb.append(t)
            for jt in range(NJ):      # vT: (j, 8*65) with ones column per head
                ps = ps_mm.tile([128, 512], F32, tag="mm")
                for ct in range(NCT):
                    nc.tensor.matmul(ps[:, :],
                                     (xn_sb[ct][:, jt * 128:(jt + 1) * 128]),
                                     (wq_sb[ct][:, 2 * HID:3 * HID]),
                                     start=(ct == 0), stop=(ct == NCT - 1))
                t = vtp.tile([128, H * (D + 1)], F32R, tag="vt")
                tv = t[:].rearrange("p (h e) -> p h e", e=D + 1)
                nc.vector.tensor_copy(tv[:, :, 0:D],
                                      ps[:, :].rearrange("p (h e) -> p h e", e=D))
                nc.vector.tensor_copy(
                    tv[:, :, D:D + 1].rearrange("p h o -> p (h o)"),
                    stage_col[:, 0:H])
                vt_sb.append(t)

        # ---------------- attention ----------------
        ysb = [yp.tile([128, LQ], F32R, tag="y", name=f"ysb{i}")
               for i in range(4)]
        inv_sb = []
        with tc.tile_pool(name="ps_sim", bufs=2, space="PSUM") as ps_sim, \
                tc.tile_pool(name="ps_y", bufs=2, space="PSUM") as ps_y:
            for h in range(H):
                mt, po = h // 2, (h % 2) * D
                yps = ps_y.tile([D + 1, LQ], F32, tag="ypsum")
                for jt in range(NJ):
                    sps = ps_sim.tile([128, LQ], F32, tag="sim")
                    for n in range(LQ // 512):
                        nc.tensor.matmul(
                            sps[:, n * 512:(n + 1) * 512],
                            (k_sb[mt][po:po + D, jt * 128:(jt + 1) * 128]),
                            (q_sb[mt][po:po + D, n * 512:(n + 1) * 512]),
                            start=True, stop=True)
                    pt = ptp.tile([128, LQ], F32R, tag="pt")
                    nc.scalar.activation(pt[:], sps[:, :], AF.Exp)
                    for n in range(LQ // 512):
                        nc.tensor.matmul(
                            yps[:, n * 512:(n + 1) * 512],
                            (vt_sb[jt][:, h * (D + 1):(h + 1) * (D + 1)]),
                            (pt[:, n * 512:(n + 1) * 512]),
                            start=(jt == 0), stop=(jt == NJ - 1))
                nc.vector.tensor_copy(ysb[mt][po:po + D, :], yps[0:D, :])
                inv = invp.tile([1, LQ], F32R, tag=f"inv{h}")
                nc.vector.reciprocal(inv[:], yps[D:D + 1, :])
                inv_sb.append(inv)

        # ---------------- normalize y by denominator ----------------
        with tc.tile_pool(name="ps_bc", bufs=2, space="PSUM") as ps_bc:
            for hp in range(4):
                bcp = ps_bc.tile([128, LQ], F32, tag="bcp")
                for n in range(LQ // 512):
                    sl = slice(n * 512, (n + 1) * 512)
                    nc.tensor.matmul(bcp[:, sl], (colsel[0][:]),
                                     (inv_sb[2 * hp][0:1, sl]),
                                     start=True, stop=False)
                    nc.tensor.matmul(bcp[:, sl], (colsel[1][:]),
                                     (inv_sb[2 * hp + 1][0:1, sl]),
                                     start=False, stop=True)
                nc.vector.tensor_mul(ysb[hp][:], ysb[hp][:], bcp[:, :])

        # ---------------- output projection + final rms-norm ----------------
        with tc.tile_pool(name="ps_out", bufs=2, space="PSUM") as ps_out:
            out_sb = []
            for mt in range(2):
                ps = ps_out.tile([128, LQ], F32, tag="ops")
                for n in range(LQ // 512):
                    for kt in range(4):
                        nc.tensor.matmul(
                            ps[:, n * 512:(n + 1) * 512],
                            (wo_sb[kt][:, mt * 128:(mt + 1) * 128]),
                            (ysb[kt][:, n * 512:(n + 1) * 512]),
                            start=(kt == 0), stop=(kt == 3))
                t = outp.tile([128, LQ], F32, tag="osb")
                nc.vector.tensor_scalar_add(t[:], ps[:, :], bo_sb[mt][:])
                out_sb.append(t)

            ss2 = ps_out.tile([1, LQ], F32, tag="ss2", bufs=1)
            for n in range(LQ // 512):
                for mt in range(2):
                    sq = ptp.tile([128, 512], F32R, tag="pt")
                    os = out_sb[mt][:, n * 512:(n + 1) * 512]
                    nc.vector.tensor_mul(sq[:], os, os)
                    nc.tensor.matmul(ss2[0:1, n * 512:(n + 1) * 512],
                                     (ones_col[:]), (sq[:]),
                                     start=(mt == 0), stop=(mt == 1))
            s2tmp = widep.tile([1, LQ], F32, tag="wide")
            nc.scalar.activation(s2tmp[:], ss2[0:1, :], AF.Sqrt,
                                 bias=eps_t[:], scale=1.0 / C)
            s2 = widep.tile([1, LQ], F32R, tag="wide")
            nc.vector.reciprocal(s2[:], s2tmp[:])
            bc2 = ps_out.tile([128, LQ], F32, tag="bc2", bufs=1)
            for n in range(LQ // 512):
                nc.tensor.matmul(bc2[:, n * 512:(n + 1) * 512],
                                 (ones_row[:]), (s2[0:1, n * 512:(n + 1) * 512]),
                                 start=True, stop=True)
            for mt in range(2):
                t = finp.tile([128, LQ], F32, tag="fin")
                nc.vector.scalar_tensor_tensor(
                    t[:], out_sb[mt][:], g2_sb[mt][:], bc2[:, :],
                    op0=mybir.AluOpType.mult, op1=mybir.AluOpType.mult)
                nc.sync.dma_start(out[mt * 128:(mt + 1) * 128, :], t[:])


_NC = None


def _get_nc():
    global _NC
    if _NC is None:
        nc = bacc.Bacc("TRN2", target_bir_lowering=False, debug=False,
                       enable_asserts=False, num_devices=8)
        x_d = nc.dram_tensor("x", [C, L], F32, kind="ExternalInput")
        wq_d = nc.dram_tensor("wqkvT", [C, 3 * HID], F32R, kind="ExternalInput")
        wo_d = nc.dram_tensor("woutT", [HID, C], F32R, kind="ExternalInput")
        b_d = nc.dram_tensor("bout", [C, 1], F32, kind="ExternalInput")
        g2_d = nc.dram_tensor("g2v", [C, 1], F32, kind="ExternalInput")
        out_d = nc.dram_tensor("out", [C, LQ], F32, kind="ExternalOutput")
        with tile.TileContext(nc) as tc:
            _body(tc, x_d.ap(), wq_d.ap(), wo_d.ap(), b_d.ap(), g2_d.ap(),
                  out_d.ap())
        nc.compile()
        _NC = nc
    return _NC


def _in_maps(x, g1, w_qkv, w_out, b_out, g2):
    w2 = (np.asarray(w_qkv, np.float32) * np.asarray(g1, np.float32).reshape(1, C))
    w2[:HID] *= D ** -0.5
    wqkvT = np.ascontiguousarray(w2.T, np.float32)
    woutT = np.ascontiguousarray(np.asarray(w_out, np.float32).T)
    bo = np.asarray(b_out, np.float32).reshape(C, 1)
    g2v = np.asarray(g2, np.float32).reshape(C, 1)
    maps = []
    for core in range(8):
        b, half = divmod(core, 2)
        xb = np.asarray(x[b], np.float32)
        x_core = np.ascontiguousarray(np.concatenate(
            [xb[:, half * LQ:(half + 1) * LQ],
             xb[:, (1 - half) * LQ:(2 - half) * LQ]], axis=1))
        maps.append({"x": x_core, "wqkvT": wqkvT, "woutT": woutT,
                     "bout": bo, "g2v": g2v})
    return maps


def _assemble(results):
    out = np.empty((B, C, L), np.float32)
    for core in range(8):
        b, half = divmod(core, 2)
        out[b][:, half * LQ:(half + 1) * LQ] = results[core]["out"]
    return out


def kernel(x, g1, w_qkv, w_out, b_out, g2, _trace=False, _tmpdir=None):
    res = run_bass_kernel_spmd(_get_nc(), _in_maps(x, g1, w_qkv, w_out, b_out, g2),
                               core_ids=list(range(8)), trace=_trace,
                               tmpdir=_tmpdir)
    out = _assemble(res.results)
    if _trace:
        return out, res
    return out


# revision 17
# speedup vs baseline: 370.0232x; 370.0232x over previous
"""Trainium2 Bass kernel for the attention module (b=4, c=256, l=2048, h=8, d=64).

Sharding: 8 cores = 4 batches x 2 query-halves. Each core receives its batch's
x with columns permuted so its own query half comes first; it computes k/v for
all 2048 key positions and the attention output for its 1024 queries, then the
output projection + final rms-norm for its slice. Output slices are disjoint,
so no collectives are needed and the host just reassembles.

Device algorithm per core (all fp32, matmuls in fp32r):
  ss_j   = sum_c x[c,j]^2                      (PE ones-reduce)
  s_j    = 1/sqrt(ss_j/256 + eps)              (ACT sqrt + DVE reciprocal)
  xn     = x * bcast(s)                        (PE ones-broadcast + DVE)
  q,k    = (wqkvT-slices).T @ xn               (g1, 1/sqrt(d) folded on host)
  vT     = xn_slice.T @ wqkvT[:,2H:3H]         (v transposed, + ones column)
  per head:  simT = k_h.T @ q_h  (j on partitions, i free)
             p = exp(simT)                     (no max-subtract: logits bounded)
             [y_h; den] += [v_h | 1].T @ p     (denominator fused as column 65)
  y_h   /= bcast(den_h)                        (colsel matmul bcast + DVE)
  out    = woutT.T @ y + b_out
  result = out * bcast(1/sqrt(ss2/256+eps)) * g2
"""
import os
import sys

import numpy as np

if "/opt/trn_rl_repo" not in sys.path:
    sys.path.insert(0, "/opt/trn_rl_repo")

import concourse.bass as bass  # noqa: E402
import concourse.tile as tile  # noqa: E402
from concourse import bacc, mybir  # noqa: E402
from concourse.bass_utils import run_bass_kernel_spmd  # noqa: E402

F32 = mybir.dt.float32
F32R = mybir.dt.float32r
AF = mybir.ActivationFunctionType

B, C, L = 4, 256, 2048
H, D = 8, 64
HID = H * D
LQ = L // 2      # queries per core
NCT = C // 128   # 2 c-tiles
NJ = L // 128    # 16 j-tiles
EPS_B = 1e-26    # bias inside sqrt(ss/256 + eps): clamps like max(n,1e-12)


def _body(tc, x, wq, wo, bo, g2, out):
    nc = tc.nc
    from contextlib import ExitStack
    with ExitStack() as ctx:
        ctx.enter_context(nc.allow_low_precision(
            reason="fp32r matmul operands are rounded by design"))
        const = ctx.enter_context(tc.tile_pool(name="const", bufs=1))
        big = ctx.enter_context(tc.tile_pool(name="big", bufs=4))
        xnp = ctx.enter_context(tc.tile_pool(name="xn", bufs=2))
        qp = ctx.enter_context(tc.tile_pool(name="q", bufs=4))
        vtp = ctx.enter_context(tc.tile_pool(name="vt", bufs=16))
        ptp = ctx.enter_context(tc.tile_pool(name="pt", bufs=3))
        yp = ctx.enter_context(tc.tile_pool(name="y", bufs=4))
        invp = ctx.enter_context(tc.tile_pool(name="inv", bufs=1))
        widep = ctx.enter_context(tc.tile_pool(name="wide", bufs=2))
        outp = ctx.enter_context(tc.tile_pool(name="outp", bufs=2))
        finp = ctx.enter_context(tc.tile_pool(name="fin", bufs=2))

        # ---------------- constants & weights ----------------
        # memset cannot write float32r; stage in f32 and copy (dtype-convert).
        stage_col = const.tile([128, 8], F32, tag="stage_col")
        nc.vector.memset(stage_col[:], 1.0)
        stage_row = const.tile([1, 128], F32, tag="stage_row")
        nc.vector.memset(stage_row[:], 1.0)
        ones_col = const.tile([128, 1], F32R, tag="ones_col")
        nc.vector.tensor_copy(ones_col[:], stage_col[:, 0:1])
        ones_row = const.tile([1, 128], F32R, tag="ones_row")
        nc.vector.tensor_copy(ones_row[:], stage_row[:])
        eps_t = const.tile([1, 1], F32, tag="eps")
        nc.vector.memset(eps_t[:], EPS_B)
        stage_sel = []
        for i in range(2):
            t = const.tile([1, 128], F32, tag=f"stage_sel{i}")
            nc.vector.memset(t[:, 0:64], 1.0 if i == 0 else 0.0)
            nc.vector.memset(t[:, 64:128], 0.0 if i == 0 else 1.0)
            stage_sel.append(t)
        colsel = []
        for i in range(2):
            t = const.tile([1, 128], F32R, tag=f"colsel{i}")
            nc.vector.tensor_copy(t[:], stage_sel[i][:])
            colsel.append(t)

        wq_sb = []
        for ct in range(NCT):
            t = const.tile([128, 3 * HID], F32R, tag=f"wq{ct}")
            nc.sync.dma_start(t[:], wq[ct * 128:(ct + 1) * 128, :])
            wq_sb.append(t)
        wo_sb = []
        for kt in range(4):
            t = const.tile([128, C], F32R, tag=f"wo{kt}")
            nc.sync.dma_start(t[:], wo[kt * 128:(kt + 1) * 128, :])
            wo_sb.append(t)
        bo_sb, g2_sb = [], []
        for mt in range(2):
            tb = const.tile([128, 1], F32, tag=f"bo{mt}")
            nc.sync.dma_start(tb[:], bo[mt * 128:(mt + 1) * 128, :])
            bo_sb.append(tb)
            tg = const.tile([128, 1], F32, tag=f"g2{mt}")
            nc.sync.dma_start(tg[:], g2[mt * 128:(mt + 1) * 128, :])
            g2_sb.append(tg)

        x_sb = []
        for ct in range(NCT):
            t = big.tile([128, L], F32, tag="big")
            for n in range(4):
                nc.sync.dma_start(t[:, n * 512:(n + 1) * 512],
                                  x[ct * 128:(ct + 1) * 128,
                                    n * 512:(n + 1) * 512])
            x_sb.append(t)

        # ---------------- input rms-norm ----------------
        with tc.tile_pool(name="ps_pre", bufs=1, space="PSUM") as ps_pre:
            ss = ps_pre.tile([1, L], F32, tag="ss")
            for n in range(L // 512):
                for ct in range(NCT):
                    sq = ptp.tile([128, 512], F32R, tag="pt")
                    xs = x_sb[ct][:, n * 512:(n + 1) * 512]
                    nc.vector.tensor_mul(sq[:], xs, xs)
                    nc.tensor.matmul(ss[0:1, n * 512:(n + 1) * 512],
                                     ones_col[:], sq[:],
                                     start=(ct == 0), stop=(ct == NCT - 1))
            stmp = widep.tile([1, L], F32, tag="wide")
            nc.scalar.activation(stmp[:], ss[0:1, :], AF.Sqrt,
                                 bias=eps_t[:], scale=1.0 / C)
            s1 = widep.tile([1, L], F32R, tag="wide")
            nc.vector.reciprocal(s1[:], stmp[:])
            bc_in = ps_pre.tile([128, L], F32, tag="bc_in")
            for n in range(L // 512):
                nc.tensor.matmul(bc_in[:, n * 512:(n + 1) * 512],
                                 ones_row[:], s1[0:1, n * 512:(n + 1) * 512],
                                 start=True, stop=True)
            xn_sb = []
            for ct in range(NCT):
                t = xnp.tile([128, L], F32R, tag="xn")
                nc.vector.tensor_mul(t[:], x_sb[ct][:], bc_in[:, :])
                xn_sb.append(t)

        # ---------------- qkv projections ----------------
        # order: q0, k0 first, then all vT, then remaining q/k so the first
        # attention heads (which need only q0/k0/vT) start as early as possible
        q_sb, k_sb, vt_sb = [None] * 4, [None] * 4, []
        with tc.tile_pool(name="ps_mm", bufs=3, space="PSUM") as ps_mm:
            def make_q(mt):
                ps = ps_mm.tile([128, LQ], F32, tag="mm", name=f"qps{mt}")
                for n in range(LQ // 512):
                    for ct in range(NCT):
                        nc.tensor.matmul(
                            ps[:, n * 512:(n + 1) * 512],
                            wq_sb[ct][:, mt * 128:(mt + 1) * 128],
                            xn_sb[ct][:, n * 512:(n + 1) * 512],
                            start=(ct == 0), stop=(ct == NCT - 1))
                t = qp.tile([128, LQ], F32R, tag="q", name=f"qsb{mt}")
                nc.vector.tensor_copy(t[:], ps[:, :])
                q_sb[mt] = t

            def make_k(mt):
                t = big.tile([128, L], F32R, tag="big", name=f"ksb{mt}")
                for half in range(2):
                    ps = ps_mm.tile([128, LQ], F32, tag="mm",
                                    name=f"kps{mt}_{half}")
                    for n in range(2):
                        nl = half * 2 + n
                        for ct in range(NCT):
                            nc.tensor.matmul(
                                ps[:, n * 512:(n + 1) * 512],
                                wq_sb[ct][:, HID + mt * 128:HID + (mt + 1) * 128],
                                xn_sb[ct][:, nl * 512:(nl + 1) * 512],
                                start=(ct == 0), stop=(ct == NCT - 1))
                    nc.vector.tensor_copy(t[:, half * LQ:(half + 1) * LQ],
                                          ps[:, :])
                k_sb[mt] = t

            make_q(0)
            make_k(0)
            for jt in range(NJ):      # vT: (j, 8*65) with ones column per head
                ps = ps_mm.tile([128, 512], F32, tag="mm", name=f"vps{jt}")
                for ct in range(NCT):
                    nc.tensor.matmul(ps[:, :],
                                     xn_sb[ct][:, jt * 128:(jt + 1) * 128],
                                     wq_sb[ct][:, 2 * HID:3 * HID],
                                     start=(ct == 0), stop=(ct == NCT - 1))
                t = vtp.tile([128, H * (D + 1)], F32R, tag="vt", name=f"vt{jt}")
                tv = t[:].rearrange("p (h e) -> p h e", e=D + 1)
                nc.vector.tensor_copy(tv[:, :, 0:D],
                                      ps[:, :].rearrange("p (h e) -> p h e", e=D))
                nc.vector.tensor_copy(
                    tv[:, :, D:D + 1].rearrange("p h o -> p (h o)"),
                    stage_col[:, 0:H])
                vt_sb.append(t)
            for mt in range(1, 4):
                make_q(mt)
                make_k(mt)

        # ---------------- attention ----------------
        ysb = [yp.tile([128, LQ], F32R, tag="y", name=f"ysb{i}")
               for i in range(4)]
        inv_sb = []
        with tc.tile_pool(name="ps_sim", bufs=2, space="PSUM") as ps_sim, \
                tc.tile_pool(name="ps_y", bufs=2, space="PSUM") as ps_y:
            for h in range(H):
                mt, po = h // 2, (h % 2) * D
                yps = ps_y.tile([D + 1, LQ], F32, tag="ypsum")
                for jt in range(NJ):
                    sps = ps_sim.tile([128, LQ], F32, tag="sim")
                    for n in range(LQ // 512):
                        nc.tensor.matmul(
                            sps[:, n * 512:(n + 1) * 512],
                            k_sb[mt][po:po + D, jt * 128:(jt + 1) * 128],
                            q_sb[mt][po:po + D, n * 512:(n + 1) * 512],
                            start=True, stop=True)
                    pt = ptp.tile([128, LQ], F32R, tag="pt")
                    nc.scalar.activation(pt[:], sps[:, :], AF.Exp)
                    for n in range(LQ // 512):
                        nc.tensor.matmul(
                            yps[:, n * 512:(n + 1) * 512],
                            vt_sb[jt][:, h * (D + 1):(h + 1) * (D + 1)],
                            pt[:, n * 512:(n + 1) * 512],
                            start=(jt == 0), stop=(jt == NJ - 1))
                nc.vector.tensor_copy(ysb[mt][po:po + D, :], yps[0:D, :])
                inv = invp.tile([1, LQ], F32R, tag=f"inv{h}", name=f"inv{h}")
                nc.vector.reciprocal(inv[:], yps[D:D + 1, :])
                inv_sb.append(inv)

        # ---------------- normalize y by denominator ----------------
        with tc.tile_pool(name="ps_bc", bufs=2, space="PSUM") as ps_bc:
            for hp in range(4):
                bcp = ps_bc.tile([128, LQ], F32, tag="bcp")
                for n in range(LQ // 512):
                    sl = slice(n * 512, (n + 1) * 512)
                    nc.tensor.matmul(bcp[:, sl], colsel[0][:],
                                     inv_sb[2 * hp][0:1, sl],
                                     start=True, stop=False)
                    nc.tensor.matmul(bcp[:, sl], colsel[1][:],
                                     inv_sb[2 * hp + 1][0:1, sl],
                                     start=False, stop=True)
                nc.vector.tensor_mul(ysb[hp][:], ysb[hp][:], bcp[:, :])

        # ---------------- output projection + final rms-norm ----------------
        with tc.tile_pool(name="ps_out", bufs=2, space="PSUM") as ps_out:
            out_sb = []
            for mt in range(2):
                ps = ps_out.tile([128, LQ], F32, tag="ops")
                for n in range(LQ // 512):
                    for kt in range(4):
                        nc.tensor.matmul(
                            ps[:, n * 512:(n + 1) * 512],
                            wo_sb[kt][:, mt * 128:(mt + 1) * 128],
                            ysb[kt][:, n * 512:(n + 1) * 512],
                            start=(kt == 0), stop=(kt == 3))
                t = outp.tile([128, LQ], F32, tag="osb")
                nc.vector.tensor_scalar_add(t[:], ps[:, :], bo_sb[mt][:])
                out_sb.append(t)

            ss2 = ps_out.tile([1, LQ], F32, tag="ss2", bufs=1)
            for n in range(LQ // 512):
                for mt in range(2):
                    sq = ptp.tile([128, 512], F32R, tag="pt")
                    os_ = out_sb[mt][:, n * 512:(n + 1) * 512]
                    nc.vector.tensor_mul(sq[:], os_, os_)
                    nc.tensor.matmul(ss2[0:1, n * 512:(n + 1) * 512],
                                     ones_col[:], sq[:],
                                     start=(mt == 0), stop=(mt == 1))
            s2tmp = widep.tile([1, LQ], F32, tag="wide")
            nc.scalar.activation(s2tmp[:], ss2[0:1, :], AF.Sqrt,
                                 bias=eps_t[:], scale=1.0 / C)
            s2 = widep.tile([1, LQ], F32R, tag="wide")
            nc.vector.reciprocal(s2[:], s2tmp[:])
            bc2 = ps_out.tile([128, LQ], F32, tag="bc2", bufs=1)
            for n in range(LQ // 512):
                nc.tensor.matmul(bc2[:, n * 512:(n + 1) * 512],
                                 ones_row[:], s2[0:1, n * 512:(n + 1) * 512],
                                 start=True, stop=True)
            for mt in range(2):
                t = finp.tile([128, LQ], F32, tag="fin")
                nc.vector.scalar_tensor_tensor(
                    t[:], out_sb[mt][:], g2_sb[mt][:], bc2[:, :],
                    op0=mybir.AluOpType.mult, op1=mybir.AluOpType.mult)
                nc.sync.dma_start(out[mt * 128:(mt + 1) * 128, :], t[:])


_NC = None


def _get_nc():
    global _NC
    if _NC is None:
        nc = bacc.Bacc("TRN2", target_bir_lowering=False, debug=False,
                       enable_asserts=False, num_devices=8)
        x_d = nc.dram_tensor("x", [C, L], F32, kind="ExternalInput")
        wq_d = nc.dram_tensor("wqkvT", [C, 3 * HID], F32R, kind="ExternalInput")
        wo_d = nc.dram_tensor("woutT", [HID, C], F32R, kind="ExternalInput")
        b_d = nc.dram_tensor("bout", [C, 1], F32, kind="ExternalInput")
        g2_d = nc.dram_tensor("g2v", [C, 1], F32, kind="ExternalInput")
        out_d = nc.dram_tensor("out", [C, LQ], F32, kind="ExternalOutput")
        with tile.TileContext(nc) as tc:
            _body(tc, x_d.ap(), wq_d.ap(), wo_d.ap(), b_d.ap(), g2_d.ap(),
                  out_d.ap())
        nc.compile()
        _NC = nc
    return _NC


def _in_maps(x, g1, w_qkv, w_out, b_out, g2):
    w2 = (np.asarray(w_qkv, np.float32) * np.asarray(g1, np.float32).reshape(1, C))
    w2[:HID] *= D ** -0.5
    wqkvT = np.ascontiguousarray(w2.T, np.float32)
    woutT = np.ascontiguousarray(np.asarray(w_out, np.float32).T)
    bo = np.asarray(b_out, np.float32).reshape(C, 1)
    g2v = np.asarray(g2, np.float32).reshape(C, 1)
    maps = []
    for core in range(8):
        b, half = divmod(core, 2)
        xb = np.asarray(x[b], np.float32)
        x_core = np.ascontiguousarray(np.concatenate(
            [xb[:, half * LQ:(half + 1) * LQ],
             xb[:, (1 - half) * LQ:(2 - half) * LQ]], axis=1))
        maps.append({"x": x_core, "wqkvT": wqkvT, "woutT": woutT,
                     "bout": bo, "g2v": g2v})
    return maps


def _assemble(results):
    out = np.empty((B, C, L), np.float32)
    for core in range(8):
        b, half = divmod(core, 2)
        out[b][:, half * LQ:(half + 1) * LQ] = results[core]["out"]
    return out


def kernel(x, g1, w_qkv, w_out, b_out, g2, _trace=False, _tmpdir=None):
    res = run_bass_kernel_spmd(_get_nc(), _in_maps(x, g1, w_qkv, w_out, b_out, g2),
                               core_ids=list(range(8)), trace=_trace,
                               tmpdir=_tmpdir)
    out = _assemble(res.results)
    if _trace:
        return out, res
    return out
